# revision 4
# baseline (speedup 1.0000x reference)
"""Trainium2 Bass kernel for the 2-layer GAT (nn_GAT_47459388621602).

Strategy (8 NeuronCores, SPMD, one NEFF):
  - Host does index/layout prep only: add self-loops, assign destination nodes
    to cores (degree-stratified, lo/hi-source-balanced), build per-core padded
    CSR gather index lists (int16, table split in two halves for dma_gather).
  - Device, per core:
      P1: h1/ld1 tables for ALL nodes (replicated): h1 = bn(x) @ W1 (BN folded
          into weights on host; bias handled via rank-1 + downstream folds).
      P2: edge aggregation for the core's destination slab: dma_gather of
          source rows, per-edge softmax numerators on ACT/DVE, weighted sums
          via tree-reduction, normalize, +bias, ELU -> x2 slab (transposed).
      AllGather x2 slabs across the 8 cores.
      P3: h2/ld2 tables for all nodes (replicated) from gathered x2.
      P4: layer-2 aggregation for the slab -> out2 slab [6272, 160].
  - Host re-assembles/unpermutes the 8 slabs into the full [50000, 160] output.
"""
import os

import numpy as np

import concourse.bacc as bacc
import concourse.mybir as mybir
import concourse.tile as tile
from concourse.bass import IndirectOffsetOnAxis
from concourse.bass_utils import run_bass_kernel_spmd
from concourse.library_config import mlp as mlp_library
from concourse.masks import make_identity

N_NODES = 50000
IN_F = 129
HID = 32
HEADS = 4
N_CLS = 40
NEG_SLOPE = 0.2
BN_EPS = 1e-5
NCORES = 8
BLK = 128
NBLK = 49
SLAB = NBLK * BLK           # 6272
NID = NCORES * SLAB         # 50176
HALF = NID // 2             # 25088
NEG = -1e30
K1 = 8.0
K2 = 12.0
F2 = 160                    # layer-2 feature width
TAB2 = 192                  # padded layer-2 table row (float32s)
NT = NID // BLK             # 392 node tiles
GCHUNK = 8                  # max w-columns (x128 idxs) per dma_gather call

f32 = mybir.dt.float32
i16 = mybir.dt.int16
i32 = mybir.dt.int32


# ----------------------------------------------------------------- host prep
def _prep_indices(edge_index):
    src0 = np.asarray(edge_index[0], dtype=np.int64)
    dst0 = np.asarray(edge_index[1], dtype=np.int64)
    loops = np.arange(N_NODES, dtype=np.int64)
    src = np.concatenate([src0, loops])
    dst = np.concatenate([dst0, loops])

    deg = np.bincount(dst, minlength=N_NODES)

    # greedy lo/hi source split balancing each destination's in-edge halves
    out_adj_order = np.argsort(src, kind="stable")
    dst_by_src = dst[out_adj_order]
    s_starts = np.searchsorted(src[out_adj_order], np.arange(N_NODES))
    s_ends = np.searchsorted(src[out_adj_order], np.arange(N_NODES) + 1)
    balance = np.zeros(N_NODES, dtype=np.int64)
    is_lo_node = np.zeros(N_NODES, dtype=bool)
    outdeg = s_ends - s_starts
    cap = N_NODES // 2
    n_lo = n_hi = 0
    for n in np.argsort(-outdeg, kind="stable"):
        nb = dst_by_src[s_starts[n]:s_ends[n]]
        go_lo = balance[nb].sum() <= 0
        if go_lo and n_lo >= cap:
            go_lo = False
        if (not go_lo) and n_hi >= cap:
            go_lo = True
        if go_lo:
            is_lo_node[n] = True
            balance[nb] += 1
            n_lo += 1
        else:
            balance[nb] -= 1
            n_hi += 1

    is_lo_src = is_lo_node[src]
    deglo = np.bincount(dst[is_lo_src], minlength=N_NODES)
    deghi = deg - deglo

    # degree-stratified assignment; residue slot order keeps chunk types
    # aligned across cores so slot-wise max W is tight
    GRP = 4 * BLK
    blocks = {}
    for half in range(2):
        ids = np.where(is_lo_node if half == 0 else ~is_lo_node)[0]
        ids = ids[np.argsort(-deg[ids], kind="stable")]
        n_strata = (len(ids) + GRP - 1) // GRP
        assert n_strata <= NBLK
        core_blocks = [[] for _ in range(4)]
        for s in range(n_strata):
            members = ids[s * GRP: min((s + 1) * GRP, len(ids))]
            m_sorted = members[np.argsort(-deglo[members], kind="stable")]
            chs = np.array_split(m_sorted, 4)
            for t, ch in enumerate(chs):
                core_blocks[(t - s) % 4].append((s, ch))
        for q in range(4):
            core_blocks[q].sort(key=lambda x: (x[0] // 4) * 4 + (x[0] + q) % 4)
            for b in range(NBLK):
                ch = core_blocks[q][b][1] if b < len(core_blocks[q]) else np.array([], dtype=np.int64)
                blk = ch[np.argsort(-deglo[ch], kind="stable")] if len(ch) else ch
                blocks[(half * 4 + q, b)] = blk

    node_cid = np.empty(N_NODES, dtype=np.int64)
    Wlo_qb = np.ones((NCORES, NBLK), dtype=np.int64)
    Whi_qb = np.ones((NCORES, NBLK), dtype=np.int64)
    for q in range(NCORES):
        for b in range(NBLK):
            blk = blocks[(q, b)]
            for jj, n in enumerate(blk):
                node_cid[n] = q * SLAB + b * BLK + jj
            if len(blk):
                Wlo_qb[q, b] = max(1, int(deglo[blk].max()))
                Whi_qb[q, b] = max(1, int(deghi[blk].max()))

    Wlo = Wlo_qb.max(axis=0)
    Whi = Whi_qb.max(axis=0)
    S = int((Wlo + Whi).sum())
    offs = np.zeros(NBLK + 1, dtype=np.int64)
    offs[1:] = np.cumsum(Wlo + Whi)

    idx16 = np.zeros((NCORES, BLK, S), dtype=np.int16)
    maskneg = np.full((NCORES, BLK, S), NEG, dtype=np.float32)

    eorder = np.argsort(node_cid[dst], kind="stable")
    src_cid_sorted = node_cid[src[eorder]]
    dst_cid_sorted = node_cid[dst[eorder]]
    lo_sorted = is_lo_src[eorder]
    starts = np.searchsorted(dst_cid_sorted, np.arange(NID))
    ends = np.searchsorted(dst_cid_sorted, np.arange(NID) + 1)

    for q in range(NCORES):
        qbase = q * SLAB
        for b in range(NBLK):
            o = int(offs[b])
            wl = int(Wlo[b])
            for jj in range(BLK):
                cid = qbase + b * BLK + jj
                e0, e1 = starts[cid], ends[cid]
                ss = src_cid_sorted[e0:e1]
                ll = lo_sorted[e0:e1]
                slo = ss[ll]
                shi = ss[~ll] - HALF
                idx16[q, jj, o:o + len(slo)] = slo.astype(np.int16)
                maskneg[q, jj, o:o + len(slo)] = 0.0
                idx16[q, jj, o + wl:o + wl + len(shi)] = shi.astype(np.int16)
                maskneg[q, jj, o + wl:o + wl + len(shi)] = 0.0

    # wrapped int16 gather index stream: per block, lo range then hi range,
    # each [128, W*8] ( slot-major wrapped by 16, replicated to 128 partitions )
    idxw = np.zeros((NCORES, BLK, S * 8), dtype=np.int16)
    for q in range(NCORES):
        col = 0
        for b in range(NBLK):
            o = int(offs[b])
            for (w0, w1) in ((0, int(Wlo[b])), (int(Wlo[b]), int(Wlo[b] + Whi[b]))):
                nw = w1 - w0
                sl = idx16[q, :, o + w0:o + w1].T.reshape(nw * BLK)   # slot-major
                wrapped = np.tile(sl.reshape(nw * 8, 16).T, (8, 1))   # [128, nw*8]
                idxw[q, :, col:col + nw * 8] = wrapped
                col += nw * 8
        assert col == S * 8

    ldidx = np.zeros((NCORES, BLK, NBLK), dtype=np.int32)
    for q in range(NCORES):
        for b in range(NBLK):
            ldidx[q, :, b] = q * SLAB + b * BLK + np.arange(BLK)

    return dict(node_cid=node_cid, Wlo=Wlo.astype(int), Whi=Whi.astype(int),
                offs=offs, S=S, idxw=idxw, maskneg=maskneg, ldidx=ldidx)


def _fold_weights(inp):
    g = np.asarray(inp["bn_gamma"], np.float32)
    bta = np.asarray(inp["bn_beta"], np.float32)
    mu = np.asarray(inp["bn_mean"], np.float32)
    var = np.asarray(inp["bn_var"], np.float32)
    W1 = np.asarray(inp["W1"], np.float32)
    a1s = np.asarray(inp["a1_src"], np.float32)
    a1d = np.asarray(inp["a1_dst"], np.float32)
    W2 = np.asarray(inp["W2"], np.float32)
    a2s = np.asarray(inp["a2_src"], np.float32)
    a2d = np.asarray(inp["a2_dst"], np.float32)

    s = g / np.sqrt(var + BN_EPS)
    W1p = (s[:, None] * W1).astype(np.float32)                 # [129, 128]
    b1p = ((bta - mu * s) @ W1).astype(np.float32)             # [128]
    A1s = np.zeros((HEADS * HID, HEADS), np.float32)
    A1d = np.zeros((HEADS * HID, HEADS), np.float32)
    A2s = np.zeros((HEADS * N_CLS, HEADS), np.float32)
    A2d = np.zeros((HEADS * N_CLS, HEADS), np.float32)
    for h in range(HEADS):
        A1s[h * HID:(h + 1) * HID, h] = a1s[h]
        A1d[h * HID:(h + 1) * HID, h] = a1d[h]
        A2s[h * N_CLS:(h + 1) * N_CLS, h] = a2s[h]
        A2d[h * N_CLS:(h + 1) * N_CLS, h] = a2d[h]
    W1f = np.concatenate([W1p, W1p @ A1d], axis=1)             # [129, 132]
    csd = (b1p @ A1s + b1p @ A1d).astype(np.float32)           # [4]
    W2f = np.concatenate([W2, W2 @ A2s, W2 @ A2d], axis=1)     # [128, 168]
    a1s_flat = A1s.sum(axis=1)  # not used; per-col a1s below
    return dict(W1f=W1f, b1p=b1p, csd=csd, W2f=W2f,
                a1sb=np.tile(A1s.sum(axis=1) * 0, (1, 1)))


# ----------------------------------------------------------------- program
def _build_program(Wlo, Whi, offs, S):
    PHASES = os.environ.get("GAT_PHASES", "1234")
    NB_RUN = int(os.environ.get("GAT_NBLK", str(NBLK)))
    nc = bacc.Bacc("TRN2", target_bir_lowering=False, debug=False,
                   num_devices=NCORES)

    # inputs
    t_xT = nc.dram_tensor("xT", [BLK, NID], f32, kind="ExternalInput")
    t_xl = nc.dram_tensor("xlast", [NID, 1], f32, kind="ExternalInput")
    t_W1 = nc.dram_tensor("W1f", [BLK, 132], f32, kind="ExternalInput")
    t_W1r = nc.dram_tensor("W1row", [BLK, 132], f32, kind="ExternalInput")
    t_W2 = nc.dram_tensor("W2f", [BLK, 168], f32, kind="ExternalInput")
    t_a1s = nc.dram_tensor("a1sb", [BLK, BLK], f32, kind="ExternalInput")
    t_b1p = nc.dram_tensor("b1pb", [BLK, BLK], f32, kind="ExternalInput")
    t_csd = nc.dram_tensor("csdb", [BLK, HEADS], f32, kind="ExternalInput")
    t_idxw = nc.dram_tensor("idxw", [BLK, S * 8], i16, kind="ExternalInput")
    t_mneg = nc.dram_tensor("mneg", [BLK, S], f32, kind="ExternalInput")
    t_ldix = nc.dram_tensor("ldidx", [BLK, NBLK], i32, kind="ExternalInput")
    t_out = nc.dram_tensor("out2", [SLAB, F2], f32, kind="ExternalOutput")

    with tile.TileContext(nc) as tc:
        with (
            tc.tile_pool(name="const", bufs=1) as cpool,
            tc.tile_pool(name="dram", bufs=1, space="DRAM") as dpool,
        ):
            nc.gpsimd.load_library(mlp_library)

            # internal DRAM
            h1tab = dpool.tile([NID, BLK], f32)
            ld1tab = dpool.tile([NID, HEADS], f32)
            h2tab = dpool.tile([NID, TAB2], f32)
            ld2tab = dpool.tile([NID, HEADS], f32)
            x2slabT = dpool.tile([BLK, SLAB], f32)
            x2fullT = dpool.tile([NCORES * BLK, SLAB], f32, addr_space="Shared")

            # resident constants
            W1sb = cpool.tile([BLK, 132], f32)
            nc.sync.dma_start(out=W1sb[:], in_=t_W1[:])
            W1rsb = cpool.tile([BLK, 132], f32)
            nc.sync.dma_start(out=W1rsb[:], in_=t_W1r[:])
            W2sb = cpool.tile([BLK, 168], f32)
            nc.sync.dma_start(out=W2sb[:], in_=t_W2[:])
            a1sb = cpool.tile([BLK, BLK], f32)
            nc.sync.dma_start(out=a1sb[:], in_=t_a1s[:])
            b1pb = cpool.tile([BLK, BLK], f32)
            nc.sync.dma_start(out=b1pb[:], in_=t_b1p[:])
            csdb = cpool.tile([BLK, HEADS], f32)
            nc.sync.dma_start(out=csdb[:], in_=t_csd[:])
            idxw_sb = cpool.tile([BLK, S * 8], i16)
            nc.sync.dma_start(out=idxw_sb[:], in_=t_idxw[:])
            mneg_sb = cpool.tile([BLK, S], f32)
            nc.sync.dma_start(out=mneg_sb[:], in_=t_mneg[:])
            ldix_sb = cpool.tile([BLK, NBLK], i32)
            nc.sync.dma_start(out=ldix_sb[:], in_=t_ldix[:])
            ident = cpool.tile([BLK, BLK], f32)
            make_identity(nc, ident[:])
            kb1 = cpool.tile([BLK, 1], f32)
            nc.vector.memset(kb1[:], -K1)
            kb2 = cpool.tile([BLK, 1], f32)
            nc.vector.memset(kb2[:], -K2)

            # ---------------- P1: h1 / ld1 tables
            with (
                tc.tile_pool(name="p1", bufs=3) as pool,
                tc.tile_pool(name="p1ps", bufs=2, space="PSUM") as pspool,
            ):
                for t in range(NT if "1" in PHASES else 0):
                    sl = slice(t * BLK, (t + 1) * BLK)
                    xT_t = pool.tile([BLK, BLK], f32, tag="xT")
                    nc.sync.dma_start(out=xT_t[:], in_=t_xT[:, sl])
                    xl_t = pool.tile([BLK, 1], f32, tag="xl")
                    nc.sync.dma_start(out=xl_t[:], in_=t_xl[sl, :])
                    ps = pspool.tile([BLK, 132], f32)
                    nc.tensor.matmul(out=ps[:], lhsT=xT_t[:], rhs=W1sb[:],
                                     start=True, stop=True)
                    r1 = pool.tile([BLK, 132], f32, tag="r1")
                    nc.vector.tensor_scalar_mul(out=r1[:], in0=W1rsb[:],
                                                scalar1=xl_t[:, 0:1])
                    hsb = pool.tile([BLK, 132], f32, tag="hsb")
                    nc.vector.tensor_tensor(out=hsb[:], in0=ps[:], in1=r1[:],
                                            op=mybir.AluOpType.add)
                    nc.sync.dma_start(out=h1tab[sl, :], in_=hsb[:, 0:BLK])
                    nc.sync.dma_start(out=ld1tab[sl, :], in_=hsb[:, BLK:132])

            # ---------------- P2: layer-1 aggregation -> x2slabT
            with (
                tc.tile_pool(name="p2g", bufs=2) as gpool,
                tc.tile_pool(name="p2m", bufs=2) as mpool,
                tc.tile_pool(name="p2s", bufs=3) as spool,
                tc.tile_pool(name="p2ps", bufs=2, space="PSUM") as pspool,
            ):
                for b in range(NB_RUN if "2" in PHASES else 0):
                    wl, wh = int(Wlo[b]), int(Whi[b])
                    wt = wl + wh
                    o = int(offs[b])
                    G = gpool.tile([BLK, wt * BLK], f32, tag="G")
                    G3 = G[:].rearrange("p (w f) -> p w f", f=BLK)
                    for (wbase, wlen, tab) in [(0, wl, h1tab[0:HALF, :]),
                                               (wl, wh, h1tab[HALF:NID, :])]:
                        for w0 in range(0, wlen, GCHUNK):
                            wn = min(GCHUNK, wlen - w0)
                            nc.gpsimd.dma_gather(
                                G3[:, wbase + w0:wbase + w0 + wn, :], tab,
                                idxw_sb[:, (o + wbase + w0) * 8:(o + wbase + w0 + wn) * 8],
                                wn * BLK, wn * BLK, BLK)
                    ld_t = spool.tile([BLK, HEADS], f32, tag="ld")
                    nc.gpsimd.indirect_dma_start(
                        out=ld_t[:], out_offset=None, in_=ld1tab[:],
                        in_offset=IndirectOffsetOnAxis(ap=ldix_sb[:, b:b + 1], axis=0))
                    ldc = spool.tile([BLK, HEADS], f32, tag="ldc")
                    nc.vector.tensor_tensor(out=ldc[:], in0=ld_t[:], in1=csdb[:],
                                            op=mybir.AluOpType.add)
                    # ls = sum_c G*a1s  (grouped)
                    M = mpool.tile([BLK, wt * BLK], f32, tag="M")
                    M4 = M[:].rearrange("p (w h c) -> p w h c", h=HEADS, c=HID)
                    G4 = G3.rearrange("p w (h c) -> p w h c", c=HID)
                    a1s4 = a1sb[:].rearrange("p (h c) -> p h c", c=HID).unsqueeze(1)
                    nc.vector.tensor_tensor(out=M4, in0=G4,
                                            in1=a1s4.to_broadcast([BLK, wt, HEADS, HID]),
                                            op=mybir.AluOpType.mult)
                    lst = spool.tile([BLK, wt * HEADS], f32, tag="lst")
                    lst3 = lst[:].rearrange("p (w h) -> p w h", h=HEADS)
                    nc.vector.tensor_reduce(out=lst3, in_=M4,
                                            axis=mybir.AxisListType.X,
                                            op=mybir.AluOpType.add)
                    nc.vector.tensor_tensor(
                        out=lst3, in0=lst3,
                        in1=ldc[:].unsqueeze(1).to_broadcast([BLK, wt, HEADS]),
                        op=mybir.AluOpType.add)
                    nc.vector.tensor_tensor(
                        out=lst3, in0=lst3,
                        in1=mneg_sb[:, o:o + wt].unsqueeze(2).to_broadcast([BLK, wt, HEADS]),
                        op=mybir.AluOpType.add)
                    p_t = spool.tile([BLK, wt * HEADS], f32, tag="p")
                    p3 = p_t[:].rearrange("p (w h) -> p w h", h=HEADS)
                    nc.vector.tensor_scalar_mul(out=p_t[:], in0=lst[:], scalar1=NEG_SLOPE)
                    nc.vector.tensor_tensor(out=lst[:], in0=lst[:], in1=p_t[:],
                                            op=mybir.AluOpType.max)
                    den = spool.tile([BLK, HEADS], f32, tag="den")
                    for h in range(HEADS):
                        nc.scalar.activation(out=p3[:, :, h], in_=lst3[:, :, h],
                                             func=mybir.ActivationFunctionType.Exp,
                                             bias=kb1[:, 0:1],
                                             accum_out=den[:, h:h + 1])
                    nc.vector.tensor_scalar_add(out=den[:], in0=den[:], scalar1=1e-30)
                    # M = G * p
                    nc.vector.tensor_tensor(
                        out=M4, in0=G4,
                        in1=p3.unsqueeze(3).to_broadcast([BLK, wt, HEADS, HID]),
                        op=mybir.AluOpType.mult)
                    # tree reduce over w
                    M3 = M[:].rearrange("p (w f) -> p w f", f=BLK)
                    w = wt
                    while w > 1:
                        hsz = w // 2
                        nc.vector.tensor_tensor(out=M3[:, 0:hsz, :], in0=M3[:, 0:hsz, :],
                                                in1=M3[:, w - hsz:w, :],
                                                op=mybir.AluOpType.add)
                        w -= hsz
                    rcp = spool.tile([BLK, HEADS], f32, tag="rcp")
                    nc.vector.reciprocal(out=rcp[:], in_=den[:])
                    x2 = spool.tile([BLK, BLK], f32, tag="x2")
                    x2hc = x2[:].rearrange("p (h c) -> p h c", c=HID)
                    nc.vector.tensor_tensor(
                        out=x2hc, in0=M3[:, 0, :].rearrange("p (h c) -> p h c", c=HID),
                        in1=rcp[:].unsqueeze(2).to_broadcast([BLK, HEADS, HID]),
                        op=mybir.AluOpType.mult)
                    nc.vector.tensor_tensor(out=x2[:], in0=x2[:], in1=b1pb[:],
                                            op=mybir.AluOpType.add)
                    # elu
                    ex = spool.tile([BLK, BLK], f32, tag="ex")
                    nc.scalar.activation(out=ex[:], in_=x2[:],
                                         func=mybir.ActivationFunctionType.Exp)
                    nc.vector.tensor_scalar_add(out=ex[:], in0=ex[:], scalar1=-1.0)
                    nc.vector.tensor_scalar_min(out=ex[:], in0=ex[:], scalar1=0.0)
                    nc.vector.tensor_scalar_max(out=x2[:], in0=x2[:], scalar1=0.0)
                    nc.vector.tensor_tensor(out=x2[:], in0=x2[:], in1=ex[:],
                                            op=mybir.AluOpType.add)
                    # transpose -> slab
                    tps = pspool.tile([BLK, BLK], f32, tag="tps")
                    nc.tensor.transpose(out=tps[:], in_=x2[:], identity=ident[:])
                    x2T = spool.tile([BLK, BLK], f32, tag="x2T")
                    nc.vector.tensor_copy(out=x2T[:], in_=tps[:])
                    nc.sync.dma_start(out=x2slabT[:, b * BLK:(b + 1) * BLK], in_=x2T[:])

            # ---------------- AllGather x2 slabs
            if "3" in PHASES:
              nc.gpsimd.collective_compute(
                "AllGather", mybir.AluOpType.bypass,
                replica_groups=[list(range(NCORES))],
                ins=[x2slabT.opt()], outs=[x2fullT.opt()])

            # ---------------- P3: h2 / ld2 tables
            with (
                tc.tile_pool(name="p3", bufs=3) as pool,
                tc.tile_pool(name="p3ps", bufs=2, space="PSUM") as pspool,
            ):
                for t in range(NT if "3" in PHASES else 0):
                    sl = slice(t * BLK, (t + 1) * BLK)
                    c = t // NBLK
                    i0 = (t % NBLK) * BLK
                    xt = pool.tile([BLK, BLK], f32, tag="x2t")
                    nc.sync.dma_start(out=xt[:],
                                      in_=x2fullT[c * BLK:(c + 1) * BLK, i0:i0 + BLK])
                    ps = pspool.tile([BLK, 168], f32)
                    nc.tensor.matmul(out=ps[:], lhsT=xt[:], rhs=W2sb[:],
                                     start=True, stop=True)
                    hsb = pool.tile([BLK, 168], f32, tag="h2sb")
                    nc.vector.tensor_copy(out=hsb[:], in_=ps[:])
                    nc.sync.dma_start(out=h2tab[sl, 0:164], in_=hsb[:, 0:164])
                    nc.sync.dma_start(out=ld2tab[sl, :], in_=hsb[:, 164:168])

            # ---------------- P4: layer-2 aggregation -> out2
            with (
                tc.tile_pool(name="p4g", bufs=2) as gpool,
                tc.tile_pool(name="p4m", bufs=1) as mpool,
                tc.tile_pool(name="p4s", bufs=3) as spool,
            ):
                for b in range(NB_RUN if "4" in PHASES else 0):
                    wl, wh = int(Wlo[b]), int(Whi[b])
                    wt = wl + wh
                    o = int(offs[b])
                    G = gpool.tile([BLK, wt * TAB2], f32, tag="G2")
                    G3 = G[:].rearrange("p (w f) -> p w f", f=TAB2)
                    for (wbase, wlen, tab) in [(0, wl, h2tab[0:HALF, :]),
                                               (wl, wh, h2tab[HALF:NID, :])]:
                        for w0 in range(0, wlen, GCHUNK):
                            wn = min(GCHUNK, wlen - w0)
                            nc.gpsimd.dma_gather(
                                G3[:, wbase + w0:wbase + w0 + wn, :], tab,
                                idxw_sb[:, (o + wbase + w0) * 8:(o + wbase + w0 + wn) * 8],
                                wn * BLK, wn * BLK, TAB2)
                    ld_t = spool.tile([BLK, HEADS], f32, tag="ld2")
                    nc.gpsimd.indirect_dma_start(
                        out=ld_t[:], out_offset=None, in_=ld2tab[:],
                        in_offset=IndirectOffsetOnAxis(ap=ldix_sb[:, b:b + 1], axis=0))
                    lst = spool.tile([BLK, wt * HEADS], f32, tag="lst2")
                    lst3 = lst[:].rearrange("p (w h) -> p w h", h=HEADS)
                    nc.vector.tensor_tensor(
                        out=lst3, in0=G3[:, :, F2:F2 + HEADS],
                        in1=ld_t[:].unsqueeze(1).to_broadcast([BLK, wt, HEADS]),
                        op=mybir.AluOpType.add)
                    nc.vector.tensor_tensor(
                        out=lst3, in0=lst3,
                        in1=mneg_sb[:, o:o + wt].unsqueeze(2).to_broadcast([BLK, wt, HEADS]),
                        op=mybir.AluOpType.add)
                    p_t = spool.tile([BLK, wt * HEADS], f32, tag="p2")
                    p3 = p_t[:].rearrange("p (w h) -> p w h", h=HEADS)
                    nc.vector.tensor_scalar_mul(out=p_t[:], in0=lst[:], scalar1=NEG_SLOPE)
                    nc.vector.tensor_tensor(out=lst[:], in0=lst[:], in1=p_t[:],
                                            op=mybir.AluOpType.max)
                    den = spool.tile([BLK, HEADS], f32, tag="den2")
                    for h in range(HEADS):
                        nc.scalar.activation(out=p3[:, :, h], in_=lst3[:, :, h],
                                             func=mybir.ActivationFunctionType.Exp,
                                             bias=kb2[:, 0:1],
                                             accum_out=den[:, h:h + 1])
                    nc.vector.tensor_scalar_add(out=den[:], in0=den[:], scalar1=1e-30)
                    M = mpool.tile([BLK, wt * F2], f32, tag="M2")
                    M4 = M[:].rearrange("p (w h c) -> p w h c", h=HEADS, c=N_CLS)
                    G4 = G3[:, :, 0:F2].rearrange("p w (h c) -> p w h c", c=N_CLS)
                    nc.vector.tensor_tensor(
                        out=M4, in0=G4,
                        in1=p3.unsqueeze(3).to_broadcast([BLK, wt, HEADS, N_CLS]),
                        op=mybir.AluOpType.mult)
                    M3 = M[:].rearrange("p (w f) -> p w f", f=F2)
                    w = wt
                    while w > 1:
                        hsz = w // 2
                        nc.vector.tensor_tensor(out=M3[:, 0:hsz, :], in0=M3[:, 0:hsz, :],
                                                in1=M3[:, w - hsz:w, :],
                                                op=mybir.AluOpType.add)
                        w -= hsz
                    rcp = spool.tile([BLK, HEADS], f32, tag="rcp2")
                    nc.vector.reciprocal(out=rcp[:], in_=den[:])
                    ot = spool.tile([BLK, F2], f32, tag="ot")
                    nc.vector.tensor_tensor(
                        out=ot[:].rearrange("p (h c) -> p h c", c=N_CLS),
                        in0=M3[:, 0, :].rearrange("p (h c) -> p h c", c=N_CLS),
                        in1=rcp[:].unsqueeze(2).to_broadcast([BLK, HEADS, N_CLS]),
                        op=mybir.AluOpType.mult)
                    nc.sync.dma_start(out=t_out[b * BLK:(b + 1) * BLK, :], in_=ot[:])

    nc.compile()
    return nc


_CACHE = {}


def kernel(**inputs) -> np.ndarray:
    x = np.asarray(inputs["x"], np.float32)
    P = _prep_indices(np.asarray(inputs["edge_index"]))
    node_cid = P["node_cid"]

    fw = {}
    g = np.asarray(inputs["bn_gamma"], np.float32)
    bta = np.asarray(inputs["bn_beta"], np.float32)
    mu = np.asarray(inputs["bn_mean"], np.float32)
    var = np.asarray(inputs["bn_var"], np.float32)
    W1 = np.asarray(inputs["W1"], np.float32)
    a1s = np.asarray(inputs["a1_src"], np.float32)
    a1d = np.asarray(inputs["a1_dst"], np.float32)
    W2 = np.asarray(inputs["W2"], np.float32)
    a2s = np.asarray(inputs["a2_src"], np.float32)
    a2d = np.asarray(inputs["a2_dst"], np.float32)

    s = g / np.sqrt(var + BN_EPS)
    W1p = (s[:, None] * W1).astype(np.float32)
    b1p = ((bta - mu * s) @ W1).astype(np.float32)
    A1s = np.zeros((HEADS * HID, HEADS), np.float32)
    A1d = np.zeros((HEADS * HID, HEADS), np.float32)
    A2s = np.zeros((HEADS * N_CLS, HEADS), np.float32)
    A2d = np.zeros((HEADS * N_CLS, HEADS), np.float32)
    for h in range(HEADS):
        A1s[h * HID:(h + 1) * HID, h] = a1s[h]
        A1d[h * HID:(h + 1) * HID, h] = a1d[h]
        A2s[h * N_CLS:(h + 1) * N_CLS, h] = a2s[h]
        A2d[h * N_CLS:(h + 1) * N_CLS, h] = a2d[h]
    W1f = np.concatenate([W1p, W1p @ A1d], axis=1)            # [129, 132]
    csd = (b1p @ A1s + b1p @ A1d).astype(np.float32)
    W2f = np.concatenate([W2, W2 @ A2s, W2 @ A2d], axis=1).astype(np.float32)

    # x in cid space, transposed
    xp = np.zeros((NID, IN_F), np.float32)
    xp[node_cid] = x
    xT = np.ascontiguousarray(xp[:, :BLK].T)                  # [128, NID]
    xlast = np.ascontiguousarray(xp[:, BLK:BLK + 1])          # [NID, 1]

    key = (tuple(P["Wlo"]), tuple(P["Whi"]), os.environ.get("GAT_PHASES", "1234"), os.environ.get("GAT_NBLK", ""))
    if key not in _CACHE:
        _CACHE[key] = _build_program(P["Wlo"], P["Whi"], P["offs"], P["S"])
    nc = _CACHE[key]

    common = {
        "xT": xT, "xlast": xlast,
        "W1f": np.ascontiguousarray(W1f[:BLK]),
        "W1row": np.tile(W1f[BLK:BLK + 1], (BLK, 1)),
        "W2f": W2f,
        "a1sb": np.tile(A1s.sum(axis=1)[None, :] * 0 + A1s.max(axis=1)[None, :], (BLK, 1)),
        "b1pb": np.tile(b1p[None, :], (BLK, 1)),
        "csdb": np.tile(csd[None, :], (BLK, 1)),
    }
    # a1s as a flat [128]-vector: A1s is block-diagonal; its rowwise nonzero is
    # a1_src[h, c] at column h*HID+c -> flatten:
    a1flat = np.zeros(BLK, np.float32)
    for h in range(HEADS):
        a1flat[h * HID:(h + 1) * HID] = a1s[h]
    common["a1sb"] = np.tile(a1flat[None, :], (BLK, 1))

    in_maps = []
    for q in range(NCORES):
        m = dict(common)
        m["idxw"] = np.ascontiguousarray(P["idxw"][q])
        m["mneg"] = np.ascontiguousarray(P["maskneg"][q])
        m["ldidx"] = np.ascontiguousarray(P["ldidx"][q])
        in_maps.append(m)

    res = run_bass_kernel_spmd(nc, in_maps, core_ids=list(range(NCORES)))
    outfull = np.concatenate([r["out2"] for r in res.results], axis=0)  # [NID, 160]
    return outfull[node_cid].astype(np.float32)


# revision 5
# speedup vs baseline: 1.2062x; 1.2062x over previous
"""Trainium2 Bass kernel for the 2-layer GAT (nn_GAT_47459388621602).

Strategy (8 NeuronCores, SPMD, one NEFF):
  - Host does index/layout prep only: add self-loops, assign destination nodes
    to cores (degree-stratified, lo/hi-source-balanced), build per-core padded
    CSR gather index lists (int16, table split in two halves for dma_gather).
  - Device, per core:
      P1: h1/ld1 tables for ALL nodes (replicated): h1 = bn(x) @ W1 (BN folded
          into weights on host; bias handled via rank-1 + downstream folds).
      P2: edge aggregation for the core's destination slab: dma_gather of
          source rows, per-edge softmax numerators on ACT/DVE, weighted sums
          via tree-reduction, normalize, +bias, ELU -> x2 slab (transposed).
      AllGather x2 slabs across the 8 cores.
      P3: h2/ld2 tables for all nodes (replicated) from gathered x2.
      P4: layer-2 aggregation for the slab -> out2 slab [6272, 160].
  - Host re-assembles/unpermutes the 8 slabs into the full [50000, 160] output.
"""
import os
import time

import numpy as np

import concourse.bacc as bacc
import concourse.mybir as mybir
import concourse.tile as tile
from concourse.bass import IndirectOffsetOnAxis
from concourse.bass_utils import run_bass_kernel_spmd
from concourse.library_config import mlp as mlp_library
from concourse.masks import make_identity

N_NODES = 50000
IN_F = 129
HID = 32
HEADS = 4
N_CLS = 40
NEG_SLOPE = 0.2
BN_EPS = 1e-5
NCORES = 8
BLK = 128
NBLK = 49
SLAB = NBLK * BLK           # 6272
NID = NCORES * SLAB         # 50176
HALF = NID // 2             # 25088
NEG = -1e30
K1 = 8.0
K2 = 12.0
F2 = 160                    # layer-2 feature width
TAB2 = 192                  # padded layer-2 table row (float32s)
NT = NID // BLK             # 392 node tiles
GCHUNK = 8                  # max w-columns (x128 idxs) per dma_gather call

f32 = mybir.dt.float32
i16 = mybir.dt.int16
i32 = mybir.dt.int32


# ----------------------------------------------------------------- host prep
def _prep_indices(edge_index):
    src0 = np.asarray(edge_index[0], dtype=np.int64)
    dst0 = np.asarray(edge_index[1], dtype=np.int64)
    loops = np.arange(N_NODES, dtype=np.int64)
    src = np.concatenate([src0, loops])
    dst = np.concatenate([dst0, loops])

    deg = np.bincount(dst, minlength=N_NODES)

    # greedy lo/hi source split balancing each destination's in-edge halves
    out_adj_order = np.argsort(src, kind="stable")
    dst_by_src = dst[out_adj_order]
    s_starts = np.searchsorted(src[out_adj_order], np.arange(N_NODES))
    s_ends = np.searchsorted(src[out_adj_order], np.arange(N_NODES) + 1)
    balance = np.zeros(N_NODES, dtype=np.int64)
    is_lo_node = np.zeros(N_NODES, dtype=bool)
    outdeg = s_ends - s_starts
    cap = N_NODES // 2
    n_lo = n_hi = 0
    for n in np.argsort(-outdeg, kind="stable"):
        nb = dst_by_src[s_starts[n]:s_ends[n]]
        go_lo = balance[nb].sum() <= 0
        if go_lo and n_lo >= cap:
            go_lo = False
        if (not go_lo) and n_hi >= cap:
            go_lo = True
        if go_lo:
            is_lo_node[n] = True
            balance[nb] += 1
            n_lo += 1
        else:
            balance[nb] -= 1
            n_hi += 1

    is_lo_src = is_lo_node[src]
    deglo = np.bincount(dst[is_lo_src], minlength=N_NODES)
    deghi = deg - deglo

    # degree-stratified assignment; residue slot order keeps chunk types
    # aligned across cores so slot-wise max W is tight
    GRP = 4 * BLK
    blocks = {}
    for half in range(2):
        ids = np.where(is_lo_node if half == 0 else ~is_lo_node)[0]
        ids = ids[np.argsort(-deg[ids], kind="stable")]
        n_strata = (len(ids) + GRP - 1) // GRP
        assert n_strata <= NBLK
        core_blocks = [[] for _ in range(4)]
        for s in range(n_strata):
            members = ids[s * GRP: min((s + 1) * GRP, len(ids))]
            m_sorted = members[np.argsort(-deglo[members], kind="stable")]
            chs = np.array_split(m_sorted, 4)
            for t, ch in enumerate(chs):
                core_blocks[(t - s) % 4].append((s, ch))
        for q in range(4):
            core_blocks[q].sort(key=lambda x: (x[0] // 4) * 4 + (x[0] + q) % 4)
            for b in range(NBLK):
                ch = core_blocks[q][b][1] if b < len(core_blocks[q]) else np.array([], dtype=np.int64)
                blk = ch[np.argsort(-deglo[ch], kind="stable")] if len(ch) else ch
                blocks[(half * 4 + q, b)] = blk

    node_cid = np.empty(N_NODES, dtype=np.int64)
    Wlo_qb = np.ones((NCORES, NBLK), dtype=np.int64)
    Whi_qb = np.ones((NCORES, NBLK), dtype=np.int64)
    for q in range(NCORES):
        for b in range(NBLK):
            blk = blocks[(q, b)]
            for jj, n in enumerate(blk):
                node_cid[n] = q * SLAB + b * BLK + jj
            if len(blk):
                Wlo_qb[q, b] = max(1, int(deglo[blk].max()))
                Whi_qb[q, b] = max(1, int(deghi[blk].max()))

    Wlo = Wlo_qb.max(axis=0)
    Whi = Whi_qb.max(axis=0)
    S = int((Wlo + Whi).sum())
    offs = np.zeros(NBLK + 1, dtype=np.int64)
    offs[1:] = np.cumsum(Wlo + Whi)

    idx16 = np.zeros((NCORES, BLK, S), dtype=np.int16)
    maskneg = np.full((NCORES, BLK, S), NEG, dtype=np.float32)

    eorder = np.argsort(node_cid[dst], kind="stable")
    src_cid_sorted = node_cid[src[eorder]]
    dst_cid_sorted = node_cid[dst[eorder]]
    lo_sorted = is_lo_src[eorder]
    starts = np.searchsorted(dst_cid_sorted, np.arange(NID))
    ends = np.searchsorted(dst_cid_sorted, np.arange(NID) + 1)

    for q in range(NCORES):
        qbase = q * SLAB
        for b in range(NBLK):
            o = int(offs[b])
            wl = int(Wlo[b])
            for jj in range(BLK):
                cid = qbase + b * BLK + jj
                e0, e1 = starts[cid], ends[cid]
                ss = src_cid_sorted[e0:e1]
                ll = lo_sorted[e0:e1]
                slo = ss[ll]
                shi = ss[~ll] - HALF
                idx16[q, jj, o:o + len(slo)] = slo.astype(np.int16)
                maskneg[q, jj, o:o + len(slo)] = 0.0
                idx16[q, jj, o + wl:o + wl + len(shi)] = shi.astype(np.int16)
                maskneg[q, jj, o + wl:o + wl + len(shi)] = 0.0

    # wrapped int16 gather index stream: per block, lo range then hi range,
    # each [128, W*8] ( slot-major wrapped by 16, replicated to 128 partitions )
    idxw = np.zeros((NCORES, BLK, S * 8), dtype=np.int16)
    for q in range(NCORES):
        col = 0
        for b in range(NBLK):
            o = int(offs[b])
            for (w0, w1) in ((0, int(Wlo[b])), (int(Wlo[b]), int(Wlo[b] + Whi[b]))):
                nw = w1 - w0
                sl = idx16[q, :, o + w0:o + w1].T.reshape(nw * BLK)   # slot-major
                wrapped = np.tile(sl.reshape(nw * 8, 16).T, (8, 1))   # [128, nw*8]
                idxw[q, :, col:col + nw * 8] = wrapped
                col += nw * 8
        assert col == S * 8

    ldidx = np.zeros((NCORES, BLK, NBLK), dtype=np.int32)
    for q in range(NCORES):
        for b in range(NBLK):
            ldidx[q, :, b] = q * SLAB + b * BLK + np.arange(BLK)

    return dict(node_cid=node_cid, Wlo=Wlo.astype(int), Whi=Whi.astype(int),
                offs=offs, S=S, idxw=idxw, maskneg=maskneg, ldidx=ldidx)


def _fold_weights(inp):
    g = np.asarray(inp["bn_gamma"], np.float32)
    bta = np.asarray(inp["bn_beta"], np.float32)
    mu = np.asarray(inp["bn_mean"], np.float32)
    var = np.asarray(inp["bn_var"], np.float32)
    W1 = np.asarray(inp["W1"], np.float32)
    a1s = np.asarray(inp["a1_src"], np.float32)
    a1d = np.asarray(inp["a1_dst"], np.float32)
    W2 = np.asarray(inp["W2"], np.float32)
    a2s = np.asarray(inp["a2_src"], np.float32)
    a2d = np.asarray(inp["a2_dst"], np.float32)

    s = g / np.sqrt(var + BN_EPS)
    W1p = (s[:, None] * W1).astype(np.float32)                 # [129, 128]
    b1p = ((bta - mu * s) @ W1).astype(np.float32)             # [128]
    A1s = np.zeros((HEADS * HID, HEADS), np.float32)
    A1d = np.zeros((HEADS * HID, HEADS), np.float32)
    A2s = np.zeros((HEADS * N_CLS, HEADS), np.float32)
    A2d = np.zeros((HEADS * N_CLS, HEADS), np.float32)
    for h in range(HEADS):
        A1s[h * HID:(h + 1) * HID, h] = a1s[h]
        A1d[h * HID:(h + 1) * HID, h] = a1d[h]
        A2s[h * N_CLS:(h + 1) * N_CLS, h] = a2s[h]
        A2d[h * N_CLS:(h + 1) * N_CLS, h] = a2d[h]
    W1f = np.concatenate([W1p, W1p @ A1d], axis=1)             # [129, 132]
    csd = (b1p @ A1s + b1p @ A1d).astype(np.float32)           # [4]
    W2f = np.concatenate([W2, W2 @ A2s, W2 @ A2d], axis=1)     # [128, 168]
    a1s_flat = A1s.sum(axis=1)  # not used; per-col a1s below
    return dict(W1f=W1f, b1p=b1p, csd=csd, W2f=W2f,
                a1sb=np.tile(A1s.sum(axis=1) * 0, (1, 1)))


# ----------------------------------------------------------------- program
def _build_program(Wlo, Whi, offs, S):
    PHASES = os.environ.get("GAT_PHASES", "1234")
    NB_RUN = int(os.environ.get("GAT_NBLK", str(NBLK)))
    nc = bacc.Bacc("TRN2", target_bir_lowering=False, debug=False,
                   num_devices=NCORES)

    # inputs
    t_xT = nc.dram_tensor("xT", [BLK, NID], f32, kind="ExternalInput")
    t_xl = nc.dram_tensor("xlast", [NID, 1], f32, kind="ExternalInput")
    t_W1 = nc.dram_tensor("W1f", [BLK, 132], f32, kind="ExternalInput")
    t_W1r = nc.dram_tensor("W1row", [BLK, 132], f32, kind="ExternalInput")
    t_W2 = nc.dram_tensor("W2f", [BLK, 168], f32, kind="ExternalInput")
    t_a1s = nc.dram_tensor("a1sb", [BLK, BLK], f32, kind="ExternalInput")
    t_b1p = nc.dram_tensor("b1pb", [BLK, BLK], f32, kind="ExternalInput")
    t_csd = nc.dram_tensor("csdb", [BLK, HEADS], f32, kind="ExternalInput")
    t_idxw = nc.dram_tensor("idxw", [BLK, S * 8], i16, kind="ExternalInput")
    t_mneg = nc.dram_tensor("mneg", [BLK, S], f32, kind="ExternalInput")
    t_ldix = nc.dram_tensor("ldidx", [BLK, NBLK], i32, kind="ExternalInput")
    t_out = nc.dram_tensor("out2", [SLAB, F2], f32, kind="ExternalOutput")

    with tile.TileContext(nc) as tc:
        with (
            tc.tile_pool(name="const", bufs=1) as cpool,
            tc.tile_pool(name="dram", bufs=1, space="DRAM") as dpool,
        ):
            nc.gpsimd.load_library(mlp_library)

            # internal DRAM
            h1tab = dpool.tile([NID, BLK], f32)
            ld1tab = dpool.tile([NID, HEADS], f32)
            h2tab = dpool.tile([NID, TAB2], f32)
            ld2tab = dpool.tile([NID, HEADS], f32)
            x2slabT = dpool.tile([BLK, SLAB], f32)
            x2fullT = dpool.tile([NCORES * BLK, SLAB], f32, addr_space="Shared")

            # resident constants
            W1sb = cpool.tile([BLK, 132], f32)
            nc.sync.dma_start(out=W1sb[:], in_=t_W1[:])
            W1rsb = cpool.tile([BLK, 132], f32)
            nc.sync.dma_start(out=W1rsb[:], in_=t_W1r[:])
            W2sb = cpool.tile([BLK, 168], f32)
            nc.sync.dma_start(out=W2sb[:], in_=t_W2[:])
            a1sb = cpool.tile([BLK, BLK], f32)
            nc.sync.dma_start(out=a1sb[:], in_=t_a1s[:])
            b1pb = cpool.tile([BLK, BLK], f32)
            nc.sync.dma_start(out=b1pb[:], in_=t_b1p[:])
            csdb = cpool.tile([BLK, HEADS], f32)
            nc.sync.dma_start(out=csdb[:], in_=t_csd[:])
            idxw_sb = cpool.tile([BLK, S * 8], i16)
            nc.sync.dma_start(out=idxw_sb[:], in_=t_idxw[:])
            mneg_sb = cpool.tile([BLK, S], f32)
            nc.sync.dma_start(out=mneg_sb[:], in_=t_mneg[:])
            ldix_sb = cpool.tile([BLK, NBLK], i32)
            nc.sync.dma_start(out=ldix_sb[:], in_=t_ldix[:])
            ident = cpool.tile([BLK, BLK], f32)
            make_identity(nc, ident[:])
            kb1 = cpool.tile([BLK, 1], f32)
            nc.vector.memset(kb1[:], -K1)
            kb2 = cpool.tile([BLK, 1], f32)
            nc.vector.memset(kb2[:], -K2)

            # ---------------- P1: h1 / ld1 tables
            with (
                tc.tile_pool(name="p1", bufs=3) as pool,
                tc.tile_pool(name="p1ps", bufs=2, space="PSUM") as pspool,
            ):
                for t in range(NT if "1" in PHASES else 0):
                    sl = slice(t * BLK, (t + 1) * BLK)
                    xT_t = pool.tile([BLK, BLK], f32, tag="xT")
                    nc.sync.dma_start(out=xT_t[:], in_=t_xT[:, sl])
                    xl_t = pool.tile([BLK, 1], f32, tag="xl")
                    nc.sync.dma_start(out=xl_t[:], in_=t_xl[sl, :])
                    ps = pspool.tile([BLK, 132], f32)
                    nc.tensor.matmul(out=ps[:], lhsT=xT_t[:], rhs=W1sb[:],
                                     start=True, stop=True)
                    r1 = pool.tile([BLK, 132], f32, tag="r1")
                    nc.vector.tensor_scalar_mul(out=r1[:], in0=W1rsb[:],
                                                scalar1=xl_t[:, 0:1])
                    hsb = pool.tile([BLK, 132], f32, tag="hsb")
                    nc.vector.tensor_tensor(out=hsb[:], in0=ps[:], in1=r1[:],
                                            op=mybir.AluOpType.add)
                    nc.sync.dma_start(out=h1tab[sl, :], in_=hsb[:, 0:BLK])
                    nc.sync.dma_start(out=ld1tab[sl, :], in_=hsb[:, BLK:132])

            # ---------------- P2: layer-1 aggregation -> x2slabT
            with (
                tc.tile_pool(name="p2g", bufs=2) as gpool,
                tc.tile_pool(name="p2m", bufs=2) as mpool,
                tc.tile_pool(name="p2s", bufs=3) as spool,
                tc.tile_pool(name="p2ps", bufs=2, space="PSUM") as pspool,
            ):
                for b in range(NB_RUN if "2" in PHASES else 0):
                    wl, wh = int(Wlo[b]), int(Whi[b])
                    wt = wl + wh
                    o = int(offs[b])
                    G = gpool.tile([BLK, wt * BLK], f32, tag="G")
                    G3 = G[:].rearrange("p (w f) -> p w f", f=BLK)
                    for (wbase, wlen, tab) in [(0, wl, h1tab[0:HALF, :]),
                                               (wl, wh, h1tab[HALF:NID, :])]:
                        for w0 in range(0, wlen, GCHUNK):
                            wn = min(GCHUNK, wlen - w0)
                            nc.gpsimd.dma_gather(
                                G3[:, wbase + w0:wbase + w0 + wn, :], tab,
                                idxw_sb[:, (o + wbase + w0) * 8:(o + wbase + w0 + wn) * 8],
                                wn * BLK, wn * BLK, BLK)
                    ld_t = spool.tile([BLK, HEADS], f32, tag="ld")
                    nc.gpsimd.indirect_dma_start(
                        out=ld_t[:], out_offset=None, in_=ld1tab[:],
                        in_offset=IndirectOffsetOnAxis(ap=ldix_sb[:, b:b + 1], axis=0))
                    ldc = spool.tile([BLK, HEADS], f32, tag="ldc")
                    nc.vector.tensor_tensor(out=ldc[:], in0=ld_t[:], in1=csdb[:],
                                            op=mybir.AluOpType.add)
                    # ls = sum_c G*a1s  (grouped)
                    M = mpool.tile([BLK, wt * BLK], f32, tag="M")
                    M4 = M[:].rearrange("p (w h c) -> p w h c", h=HEADS, c=HID)
                    G4 = G3.rearrange("p w (h c) -> p w h c", c=HID)
                    a1s4 = a1sb[:].rearrange("p (h c) -> p h c", c=HID).unsqueeze(1)
                    nc.vector.tensor_tensor(out=M4, in0=G4,
                                            in1=a1s4.to_broadcast([BLK, wt, HEADS, HID]),
                                            op=mybir.AluOpType.mult)
                    lst = spool.tile([BLK, wt * HEADS], f32, tag="lst")
                    lst3 = lst[:].rearrange("p (w h) -> p w h", h=HEADS)
                    nc.vector.tensor_reduce(out=lst3, in_=M4,
                                            axis=mybir.AxisListType.X,
                                            op=mybir.AluOpType.add)
                    nc.vector.tensor_tensor(
                        out=lst3, in0=lst3,
                        in1=ldc[:].unsqueeze(1).to_broadcast([BLK, wt, HEADS]),
                        op=mybir.AluOpType.add)
                    nc.vector.tensor_tensor(
                        out=lst3, in0=lst3,
                        in1=mneg_sb[:, o:o + wt].unsqueeze(2).to_broadcast([BLK, wt, HEADS]),
                        op=mybir.AluOpType.add)
                    p_t = spool.tile([BLK, wt * HEADS], f32, tag="p")
                    p3 = p_t[:].rearrange("p (w h) -> p w h", h=HEADS)
                    nc.vector.tensor_scalar_mul(out=p_t[:], in0=lst[:], scalar1=NEG_SLOPE)
                    nc.vector.tensor_tensor(out=lst[:], in0=lst[:], in1=p_t[:],
                                            op=mybir.AluOpType.max)
                    den = spool.tile([BLK, HEADS], f32, tag="den")
                    for h in range(HEADS):
                        nc.scalar.activation(out=p3[:, :, h], in_=lst3[:, :, h],
                                             func=mybir.ActivationFunctionType.Exp,
                                             bias=kb1[:, 0:1],
                                             accum_out=den[:, h:h + 1])
                    nc.vector.tensor_scalar_add(out=den[:], in0=den[:], scalar1=1e-30)
                    # M = G * p
                    nc.vector.tensor_tensor(
                        out=M4, in0=G4,
                        in1=p3.unsqueeze(3).to_broadcast([BLK, wt, HEADS, HID]),
                        op=mybir.AluOpType.mult)
                    # tree reduce over w
                    M3 = M[:].rearrange("p (w f) -> p w f", f=BLK)
                    w = wt
                    while w > 1:
                        hsz = w // 2
                        nc.vector.tensor_tensor(out=M3[:, 0:hsz, :], in0=M3[:, 0:hsz, :],
                                                in1=M3[:, w - hsz:w, :],
                                                op=mybir.AluOpType.add)
                        w -= hsz
                    rcp = spool.tile([BLK, HEADS], f32, tag="rcp")
                    nc.vector.reciprocal(out=rcp[:], in_=den[:])
                    x2 = spool.tile([BLK, BLK], f32, tag="x2")
                    x2hc = x2[:].rearrange("p (h c) -> p h c", c=HID)
                    nc.vector.tensor_tensor(
                        out=x2hc, in0=M3[:, 0, :].rearrange("p (h c) -> p h c", c=HID),
                        in1=rcp[:].unsqueeze(2).to_broadcast([BLK, HEADS, HID]),
                        op=mybir.AluOpType.mult)
                    nc.vector.tensor_tensor(out=x2[:], in0=x2[:], in1=b1pb[:],
                                            op=mybir.AluOpType.add)
                    # elu
                    ex = spool.tile([BLK, BLK], f32, tag="ex")
                    nc.scalar.activation(out=ex[:], in_=x2[:],
                                         func=mybir.ActivationFunctionType.Exp)
                    nc.vector.tensor_scalar_add(out=ex[:], in0=ex[:], scalar1=-1.0)
                    nc.vector.tensor_scalar_min(out=ex[:], in0=ex[:], scalar1=0.0)
                    nc.vector.tensor_scalar_max(out=x2[:], in0=x2[:], scalar1=0.0)
                    nc.vector.tensor_tensor(out=x2[:], in0=x2[:], in1=ex[:],
                                            op=mybir.AluOpType.add)
                    # transpose -> slab
                    tps = pspool.tile([BLK, BLK], f32, tag="tps")
                    nc.tensor.transpose(out=tps[:], in_=x2[:], identity=ident[:])
                    x2T = spool.tile([BLK, BLK], f32, tag="x2T")
                    nc.vector.tensor_copy(out=x2T[:], in_=tps[:])
                    nc.sync.dma_start(out=x2slabT[:, b * BLK:(b + 1) * BLK], in_=x2T[:])

            # ---------------- AllGather x2 slabs
            if "3" in PHASES:
              nc.gpsimd.collective_compute(
                "AllGather", mybir.AluOpType.bypass,
                replica_groups=[list(range(NCORES))],
                ins=[x2slabT.opt()], outs=[x2fullT.opt()])

            # ---------------- P3: h2 / ld2 tables
            with (
                tc.tile_pool(name="p3", bufs=3) as pool,
                tc.tile_pool(name="p3ps", bufs=2, space="PSUM") as pspool,
            ):
                for t in range(NT if "3" in PHASES else 0):
                    sl = slice(t * BLK, (t + 1) * BLK)
                    c = t // NBLK
                    i0 = (t % NBLK) * BLK
                    xt = pool.tile([BLK, BLK], f32, tag="x2t")
                    nc.sync.dma_start(out=xt[:],
                                      in_=x2fullT[c * BLK:(c + 1) * BLK, i0:i0 + BLK])
                    ps = pspool.tile([BLK, 168], f32)
                    nc.tensor.matmul(out=ps[:], lhsT=xt[:], rhs=W2sb[:],
                                     start=True, stop=True)
                    hsb = pool.tile([BLK, 168], f32, tag="h2sb")
                    nc.vector.tensor_copy(out=hsb[:], in_=ps[:])
                    nc.sync.dma_start(out=h2tab[sl, 0:164], in_=hsb[:, 0:164])
                    nc.sync.dma_start(out=ld2tab[sl, :], in_=hsb[:, 164:168])

            # ---------------- P4: layer-2 aggregation -> out2
            with (
                tc.tile_pool(name="p4g", bufs=2) as gpool,
                tc.tile_pool(name="p4m", bufs=1) as mpool,
                tc.tile_pool(name="p4s", bufs=3) as spool,
            ):
                for b in range(NB_RUN if "4" in PHASES else 0):
                    wl, wh = int(Wlo[b]), int(Whi[b])
                    wt = wl + wh
                    o = int(offs[b])
                    G = gpool.tile([BLK, wt * TAB2], f32, tag="G2")
                    G3 = G[:].rearrange("p (w f) -> p w f", f=TAB2)
                    for (wbase, wlen, tab) in [(0, wl, h2tab[0:HALF, :]),
                                               (wl, wh, h2tab[HALF:NID, :])]:
                        for w0 in range(0, wlen, GCHUNK):
                            wn = min(GCHUNK, wlen - w0)
                            nc.gpsimd.dma_gather(
                                G3[:, wbase + w0:wbase + w0 + wn, :], tab,
                                idxw_sb[:, (o + wbase + w0) * 8:(o + wbase + w0 + wn) * 8],
                                wn * BLK, wn * BLK, TAB2)
                    ld_t = spool.tile([BLK, HEADS], f32, tag="ld2")
                    nc.gpsimd.indirect_dma_start(
                        out=ld_t[:], out_offset=None, in_=ld2tab[:],
                        in_offset=IndirectOffsetOnAxis(ap=ldix_sb[:, b:b + 1], axis=0))
                    lst = spool.tile([BLK, wt * HEADS], f32, tag="lst2")
                    lst3 = lst[:].rearrange("p (w h) -> p w h", h=HEADS)
                    nc.vector.tensor_tensor(
                        out=lst3, in0=G3[:, :, F2:F2 + HEADS],
                        in1=ld_t[:].unsqueeze(1).to_broadcast([BLK, wt, HEADS]),
                        op=mybir.AluOpType.add)
                    nc.vector.tensor_tensor(
                        out=lst3, in0=lst3,
                        in1=mneg_sb[:, o:o + wt].unsqueeze(2).to_broadcast([BLK, wt, HEADS]),
                        op=mybir.AluOpType.add)
                    p_t = spool.tile([BLK, wt * HEADS], f32, tag="p2")
                    p3 = p_t[:].rearrange("p (w h) -> p w h", h=HEADS)
                    nc.vector.tensor_scalar_mul(out=p_t[:], in0=lst[:], scalar1=NEG_SLOPE)
                    nc.vector.tensor_tensor(out=lst[:], in0=lst[:], in1=p_t[:],
                                            op=mybir.AluOpType.max)
                    den = spool.tile([BLK, HEADS], f32, tag="den2")
                    for h in range(HEADS):
                        nc.scalar.activation(out=p3[:, :, h], in_=lst3[:, :, h],
                                             func=mybir.ActivationFunctionType.Exp,
                                             bias=kb2[:, 0:1],
                                             accum_out=den[:, h:h + 1])
                    nc.vector.tensor_scalar_add(out=den[:], in0=den[:], scalar1=1e-30)
                    M = mpool.tile([BLK, wt * F2], f32, tag="M2")
                    M4 = M[:].rearrange("p (w h c) -> p w h c", h=HEADS, c=N_CLS)
                    G4 = G3[:, :, 0:F2].rearrange("p w (h c) -> p w h c", c=N_CLS)
                    nc.vector.tensor_tensor(
                        out=M4, in0=G4,
                        in1=p3.unsqueeze(3).to_broadcast([BLK, wt, HEADS, N_CLS]),
                        op=mybir.AluOpType.mult)
                    M3 = M[:].rearrange("p (w f) -> p w f", f=F2)
                    w = wt
                    while w > 1:
                        hsz = w // 2
                        nc.vector.tensor_tensor(out=M3[:, 0:hsz, :], in0=M3[:, 0:hsz, :],
                                                in1=M3[:, w - hsz:w, :],
                                                op=mybir.AluOpType.add)
                        w -= hsz
                    rcp = spool.tile([BLK, HEADS], f32, tag="rcp2")
                    nc.vector.reciprocal(out=rcp[:], in_=den[:])
                    ot = spool.tile([BLK, F2], f32, tag="ot")
                    nc.vector.tensor_tensor(
                        out=ot[:].rearrange("p (h c) -> p h c", c=N_CLS),
                        in0=M3[:, 0, :].rearrange("p (h c) -> p h c", c=N_CLS),
                        in1=rcp[:].unsqueeze(2).to_broadcast([BLK, HEADS, N_CLS]),
                        op=mybir.AluOpType.mult)
                    nc.sync.dma_start(out=t_out[b * BLK:(b + 1) * BLK, :], in_=ot[:])

    nc.compile()
    return nc


_CACHE = {}


def kernel(**inputs) -> np.ndarray:
    x = np.asarray(inputs["x"], np.float32)
    P = _prep_indices(np.asarray(inputs["edge_index"]))
    node_cid = P["node_cid"]

    fw = {}
    g = np.asarray(inputs["bn_gamma"], np.float32)
    bta = np.asarray(inputs["bn_beta"], np.float32)
    mu = np.asarray(inputs["bn_mean"], np.float32)
    var = np.asarray(inputs["bn_var"], np.float32)
    W1 = np.asarray(inputs["W1"], np.float32)
    a1s = np.asarray(inputs["a1_src"], np.float32)
    a1d = np.asarray(inputs["a1_dst"], np.float32)
    W2 = np.asarray(inputs["W2"], np.float32)
    a2s = np.asarray(inputs["a2_src"], np.float32)
    a2d = np.asarray(inputs["a2_dst"], np.float32)

    s = g / np.sqrt(var + BN_EPS)
    W1p = (s[:, None] * W1).astype(np.float32)
    b1p = ((bta - mu * s) @ W1).astype(np.float32)
    A1s = np.zeros((HEADS * HID, HEADS), np.float32)
    A1d = np.zeros((HEADS * HID, HEADS), np.float32)
    A2s = np.zeros((HEADS * N_CLS, HEADS), np.float32)
    A2d = np.zeros((HEADS * N_CLS, HEADS), np.float32)
    for h in range(HEADS):
        A1s[h * HID:(h + 1) * HID, h] = a1s[h]
        A1d[h * HID:(h + 1) * HID, h] = a1d[h]
        A2s[h * N_CLS:(h + 1) * N_CLS, h] = a2s[h]
        A2d[h * N_CLS:(h + 1) * N_CLS, h] = a2d[h]
    W1f = np.concatenate([W1p, W1p @ A1d], axis=1)            # [129, 132]
    csd = (b1p @ A1s + b1p @ A1d).astype(np.float32)
    W2f = np.concatenate([W2, W2 @ A2s, W2 @ A2d], axis=1).astype(np.float32)

    # x in cid space, transposed
    xp = np.zeros((NID, IN_F), np.float32)
    xp[node_cid] = x
    xT = np.ascontiguousarray(xp[:, :BLK].T)                  # [128, NID]
    xlast = np.ascontiguousarray(xp[:, BLK:BLK + 1])          # [NID, 1]

    key = (tuple(P["Wlo"]), tuple(P["Whi"]), os.environ.get("GAT_PHASES", "1234"), os.environ.get("GAT_NBLK", ""))
    if key not in _CACHE:
        _CACHE[key] = _build_program(P["Wlo"], P["Whi"], P["offs"], P["S"])
    nc = _CACHE[key]

    common = {
        "xT": xT, "xlast": xlast,
        "W1f": np.ascontiguousarray(W1f[:BLK]),
        "W1row": np.tile(W1f[BLK:BLK + 1], (BLK, 1)),
        "W2f": W2f,
        "a1sb": np.tile(A1s.sum(axis=1)[None, :] * 0 + A1s.max(axis=1)[None, :], (BLK, 1)),
        "b1pb": np.tile(b1p[None, :], (BLK, 1)),
        "csdb": np.tile(csd[None, :], (BLK, 1)),
    }
    # a1s as a flat [128]-vector: A1s is block-diagonal; its rowwise nonzero is
    # a1_src[h, c] at column h*HID+c -> flatten:
    a1flat = np.zeros(BLK, np.float32)
    for h in range(HEADS):
        a1flat[h * HID:(h + 1) * HID] = a1s[h]
    common["a1sb"] = np.tile(a1flat[None, :], (BLK, 1))

    in_maps = []
    for q in range(NCORES):
        m = dict(common)
        m["idxw"] = np.ascontiguousarray(P["idxw"][q])
        m["mneg"] = np.ascontiguousarray(P["maskneg"][q])
        m["ldidx"] = np.ascontiguousarray(P["ldidx"][q])
        in_maps.append(m)

    t0 = time.time()
    res = run_bass_kernel_spmd(nc, in_maps, core_ids=list(range(NCORES)))
    global last_run_seconds
    last_run_seconds = time.time() - t0
    outfull = np.concatenate([r["out2"] for r in res.results], axis=0)  # [NID, 160]
    return outfull[node_cid].astype(np.float32)


last_run_seconds = None


# revision 6
# speedup vs baseline: 282.1045x; 233.8816x over previous
"""Trainium2 Bass kernel for the 2-layer GAT (nn_GAT_47459388621602).

Strategy (8 NeuronCores, SPMD, one NEFF):
  - Host does index/layout prep only: add self-loops, assign destination nodes
    to cores (degree-stratified, lo/hi-source-balanced), build per-core padded
    CSR gather index lists (int16, table split in two halves for dma_gather).
  - Device, per core:
      P1: h1/ld1 tables for ALL nodes (replicated): h1 = bn(x) @ W1 (BN folded
          into weights on host; bias handled via rank-1 + downstream folds).
      P2: edge aggregation for the core's destination slab: dma_gather of
          source rows, per-edge softmax numerators on ACT/DVE, weighted sums
          via tree-reduction, normalize, +bias, ELU -> x2 slab (transposed).
      AllGather x2 slabs across the 8 cores.
      P3: h2/ld2 tables for all nodes (replicated) from gathered x2.
      P4: layer-2 aggregation for the slab -> out2 slab [6272, 160].
  - Host re-assembles/unpermutes the 8 slabs into the full [50000, 160] output.
"""
import os
import time

import numpy as np

import concourse.bacc as bacc
import concourse.mybir as mybir
import concourse.tile as tile
from concourse.bass import IndirectOffsetOnAxis
from concourse.bass_utils import run_bass_kernel_spmd
from concourse.library_config import mlp as mlp_library
from concourse.masks import make_identity

N_NODES = 50000
IN_F = 129
HID = 32
HEADS = 4
N_CLS = 40
NEG_SLOPE = 0.2
BN_EPS = 1e-5
NCORES = 8
BLK = 128
NBLK = 49
SLAB = NBLK * BLK           # 6272
NID = NCORES * SLAB         # 50176
HALF = NID // 2             # 25088
NEG = -1e30
K1 = 8.0
K2 = 12.0
F2 = 160                    # layer-2 feature width
TAB2 = 192                  # padded layer-2 table row (float32s)
NT = NID // BLK             # 392 node tiles
GCHUNK = 8                  # max w-columns (x128 idxs) per dma_gather call

f32 = mybir.dt.float32
i16 = mybir.dt.int16
i32 = mybir.dt.int32


# ----------------------------------------------------------------- host prep
def _prep_indices(edge_index):
    src0 = np.asarray(edge_index[0], dtype=np.int64)
    dst0 = np.asarray(edge_index[1], dtype=np.int64)
    loops = np.arange(N_NODES, dtype=np.int64)
    src = np.concatenate([src0, loops])
    dst = np.concatenate([dst0, loops])

    deg = np.bincount(dst, minlength=N_NODES)

    # greedy lo/hi source split balancing each destination's in-edge halves
    out_adj_order = np.argsort(src, kind="stable")
    dst_by_src = dst[out_adj_order]
    s_starts = np.searchsorted(src[out_adj_order], np.arange(N_NODES))
    s_ends = np.searchsorted(src[out_adj_order], np.arange(N_NODES) + 1)
    balance = np.zeros(N_NODES, dtype=np.int64)
    is_lo_node = np.zeros(N_NODES, dtype=bool)
    outdeg = s_ends - s_starts
    cap = N_NODES // 2
    n_lo = n_hi = 0
    for n in np.argsort(-outdeg, kind="stable"):
        nb = dst_by_src[s_starts[n]:s_ends[n]]
        go_lo = balance[nb].sum() <= 0
        if go_lo and n_lo >= cap:
            go_lo = False
        if (not go_lo) and n_hi >= cap:
            go_lo = True
        if go_lo:
            is_lo_node[n] = True
            balance[nb] += 1
            n_lo += 1
        else:
            balance[nb] -= 1
            n_hi += 1

    is_lo_src = is_lo_node[src]
    deglo = np.bincount(dst[is_lo_src], minlength=N_NODES)
    deghi = deg - deglo

    # degree-stratified assignment; residue slot order keeps chunk types
    # aligned across cores so slot-wise max W is tight
    GRP = 4 * BLK
    blocks = {}
    for half in range(2):
        ids = np.where(is_lo_node if half == 0 else ~is_lo_node)[0]
        ids = ids[np.argsort(-deg[ids], kind="stable")]
        n_strata = (len(ids) + GRP - 1) // GRP
        assert n_strata <= NBLK
        core_blocks = [[] for _ in range(4)]
        for s in range(n_strata):
            members = ids[s * GRP: min((s + 1) * GRP, len(ids))]
            m_sorted = members[np.argsort(-deglo[members], kind="stable")]
            chs = np.array_split(m_sorted, 4)
            for t, ch in enumerate(chs):
                core_blocks[(t - s) % 4].append((s, ch))
        for q in range(4):
            core_blocks[q].sort(key=lambda x: (x[0] // 4) * 4 + (x[0] + q) % 4)
            for b in range(NBLK):
                ch = core_blocks[q][b][1] if b < len(core_blocks[q]) else np.array([], dtype=np.int64)
                blk = ch[np.argsort(-deglo[ch], kind="stable")] if len(ch) else ch
                blocks[(half * 4 + q, b)] = blk

    node_cid = np.empty(N_NODES, dtype=np.int64)
    Wlo_qb = np.ones((NCORES, NBLK), dtype=np.int64)
    Whi_qb = np.ones((NCORES, NBLK), dtype=np.int64)
    for q in range(NCORES):
        for b in range(NBLK):
            blk = blocks[(q, b)]
            for jj, n in enumerate(blk):
                node_cid[n] = q * SLAB + b * BLK + jj
            if len(blk):
                Wlo_qb[q, b] = max(1, int(deglo[blk].max()))
                Whi_qb[q, b] = max(1, int(deghi[blk].max()))

    Wlo = Wlo_qb.max(axis=0)
    Whi = Whi_qb.max(axis=0)
    S = int((Wlo + Whi).sum())
    offs = np.zeros(NBLK + 1, dtype=np.int64)
    offs[1:] = np.cumsum(Wlo + Whi)

    idx16 = np.zeros((NCORES, BLK, S), dtype=np.int16)
    maskneg = np.full((NCORES, BLK, S), NEG, dtype=np.float32)

    eorder = np.argsort(node_cid[dst], kind="stable")
    src_cid_sorted = node_cid[src[eorder]]
    dst_cid_sorted = node_cid[dst[eorder]]
    lo_sorted = is_lo_src[eorder]
    starts = np.searchsorted(dst_cid_sorted, np.arange(NID))
    ends = np.searchsorted(dst_cid_sorted, np.arange(NID) + 1)

    for q in range(NCORES):
        qbase = q * SLAB
        for b in range(NBLK):
            o = int(offs[b])
            wl = int(Wlo[b])
            for jj in range(BLK):
                cid = qbase + b * BLK + jj
                e0, e1 = starts[cid], ends[cid]
                ss = src_cid_sorted[e0:e1]
                ll = lo_sorted[e0:e1]
                slo = ss[ll]
                shi = ss[~ll] - HALF
                idx16[q, jj, o:o + len(slo)] = slo.astype(np.int16)
                maskneg[q, jj, o:o + len(slo)] = 0.0
                idx16[q, jj, o + wl:o + wl + len(shi)] = shi.astype(np.int16)
                maskneg[q, jj, o + wl:o + wl + len(shi)] = 0.0

    # wrapped int16 gather index stream: per block, lo range then hi range,
    # each [128, W*8] ( slot-major wrapped by 16, replicated to 128 partitions )
    idxw = np.zeros((NCORES, BLK, S * 8), dtype=np.int16)
    for q in range(NCORES):
        col = 0
        for b in range(NBLK):
            o = int(offs[b])
            for (w0, w1) in ((0, int(Wlo[b])), (int(Wlo[b]), int(Wlo[b] + Whi[b]))):
                nw = w1 - w0
                sl = idx16[q, :, o + w0:o + w1].T.reshape(nw * BLK)   # slot-major
                wrapped = np.tile(sl.reshape(nw * 8, 16).T, (8, 1))   # [128, nw*8]
                idxw[q, :, col:col + nw * 8] = wrapped
                col += nw * 8
        assert col == S * 8

    ldidx = np.zeros((NCORES, BLK, NBLK), dtype=np.int32)
    for q in range(NCORES):
        for b in range(NBLK):
            ldidx[q, :, b] = q * SLAB + b * BLK + np.arange(BLK)

    return dict(node_cid=node_cid, Wlo=Wlo.astype(int), Whi=Whi.astype(int),
                offs=offs, S=S, idxw=idxw, maskneg=maskneg, ldidx=ldidx)


# ----------------------------------------------------------------- program
def _build_program(Wlo, Whi, offs, S):
    PHASES = os.environ.get("GAT_PHASES", "1234")
    NB_RUN = int(os.environ.get("GAT_NBLK", str(NBLK)))
    nc = bacc.Bacc("TRN2", target_bir_lowering=False, debug=False,
                   num_devices=NCORES)

    # inputs
    t_xT = nc.dram_tensor("xT", [BLK, NID], f32, kind="ExternalInput")
    t_xl = nc.dram_tensor("xlast", [NID, 1], f32, kind="ExternalInput")
    t_W1 = nc.dram_tensor("W1f", [BLK, 132], f32, kind="ExternalInput")
    t_W1r = nc.dram_tensor("W1row", [BLK, 132], f32, kind="ExternalInput")
    t_W2 = nc.dram_tensor("W2f", [BLK, 168], f32, kind="ExternalInput")
    t_a1s = nc.dram_tensor("a1sb", [BLK, BLK], f32, kind="ExternalInput")
    t_b1p = nc.dram_tensor("b1pb", [BLK, BLK], f32, kind="ExternalInput")
    t_csd = nc.dram_tensor("csdb", [BLK, HEADS], f32, kind="ExternalInput")
    t_idxw = nc.dram_tensor("idxw", [BLK, S * 8], i16, kind="ExternalInput")
    t_mneg = nc.dram_tensor("mneg", [BLK, S], f32, kind="ExternalInput")
    t_ldix = nc.dram_tensor("ldidx", [BLK, NBLK], i32, kind="ExternalInput")
    t_out = nc.dram_tensor("out2", [SLAB, F2], f32, kind="ExternalOutput")

    with tile.TileContext(nc) as tc:
        with (
            tc.tile_pool(name="const", bufs=1) as cpool,
            tc.tile_pool(name="dram", bufs=1, space="DRAM") as dpool,
        ):
            nc.gpsimd.load_library(mlp_library)

            # internal DRAM
            h1tab = dpool.tile([NID, BLK], f32)
            ld1tab = dpool.tile([NID, HEADS], f32)
            h2tab = dpool.tile([NID, TAB2], f32)
            ld2tab = dpool.tile([NID, HEADS], f32)
            x2slabT = dpool.tile([BLK, SLAB], f32)
            x2fullT = dpool.tile([NCORES * BLK, SLAB], f32, addr_space="Shared")

            # resident constants
            W1sb = cpool.tile([BLK, 132], f32)
            nc.sync.dma_start(out=W1sb[:], in_=t_W1[:])
            W1rsb = cpool.tile([BLK, 132], f32)
            nc.sync.dma_start(out=W1rsb[:], in_=t_W1r[:])
            W2sb = cpool.tile([BLK, 168], f32)
            nc.sync.dma_start(out=W2sb[:], in_=t_W2[:])
            a1sb = cpool.tile([BLK, BLK], f32)
            nc.sync.dma_start(out=a1sb[:], in_=t_a1s[:])
            b1pb = cpool.tile([BLK, BLK], f32)
            nc.sync.dma_start(out=b1pb[:], in_=t_b1p[:])
            csdb = cpool.tile([BLK, HEADS], f32)
            nc.sync.dma_start(out=csdb[:], in_=t_csd[:])
            idxw_sb = cpool.tile([BLK, S * 8], i16)
            nc.sync.dma_start(out=idxw_sb[:], in_=t_idxw[:])
            mneg_sb = cpool.tile([BLK, S], f32)
            nc.sync.dma_start(out=mneg_sb[:], in_=t_mneg[:])
            ldix_sb = cpool.tile([BLK, NBLK], i32)
            nc.sync.dma_start(out=ldix_sb[:], in_=t_ldix[:])
            ident = cpool.tile([BLK, BLK], f32)
            make_identity(nc, ident[:])
            kb1 = cpool.tile([BLK, 1], f32)
            nc.vector.memset(kb1[:], -K1)
            kb2 = cpool.tile([BLK, 1], f32)
            nc.vector.memset(kb2[:], -K2)

            # ---------------- P1: h1 / ld1 tables
            with (
                tc.tile_pool(name="p1", bufs=3) as pool,
                tc.tile_pool(name="p1ps", bufs=2, space="PSUM") as pspool,
            ):
                for t in range(NT if "1" in PHASES else 0):
                    sl = slice(t * BLK, (t + 1) * BLK)
                    xT_t = pool.tile([BLK, BLK], f32, tag="xT")
                    nc.sync.dma_start(out=xT_t[:], in_=t_xT[:, sl])
                    xl_t = pool.tile([BLK, 1], f32, tag="xl")
                    nc.sync.dma_start(out=xl_t[:], in_=t_xl[sl, :])
                    ps = pspool.tile([BLK, 132], f32)
                    nc.tensor.matmul(out=ps[:], lhsT=xT_t[:], rhs=W1sb[:],
                                     start=True, stop=True)
                    r1 = pool.tile([BLK, 132], f32, tag="r1")
                    nc.vector.tensor_scalar_mul(out=r1[:], in0=W1rsb[:],
                                                scalar1=xl_t[:, 0:1])
                    hsb = pool.tile([BLK, 132], f32, tag="hsb")
                    nc.vector.tensor_tensor(out=hsb[:], in0=ps[:], in1=r1[:],
                                            op=mybir.AluOpType.add)
                    nc.sync.dma_start(out=h1tab[sl, :], in_=hsb[:, 0:BLK])
                    nc.sync.dma_start(out=ld1tab[sl, :], in_=hsb[:, BLK:132])

            # ---------------- P2: layer-1 aggregation -> x2slabT
            with (
                tc.tile_pool(name="p2g", bufs=2) as gpool,
                tc.tile_pool(name="p2m", bufs=2) as mpool,
                tc.tile_pool(name="p2s", bufs=3) as spool,
                tc.tile_pool(name="p2ps", bufs=2, space="PSUM") as pspool,
            ):
                for b in range(NB_RUN if "2" in PHASES else 0):
                    wl, wh = int(Wlo[b]), int(Whi[b])
                    wt = wl + wh
                    o = int(offs[b])
                    G = gpool.tile([BLK, wt * BLK], f32, tag="G")
                    G3 = G[:].rearrange("p (w f) -> p w f", f=BLK)
                    for (wbase, wlen, tab) in [(0, wl, h1tab[0:HALF, :]),
                                               (wl, wh, h1tab[HALF:NID, :])]:
                        for w0 in range(0, wlen, GCHUNK):
                            wn = min(GCHUNK, wlen - w0)
                            nc.gpsimd.dma_gather(
                                G3[:, wbase + w0:wbase + w0 + wn, :], tab,
                                idxw_sb[:, (o + wbase + w0) * 8:(o + wbase + w0 + wn) * 8],
                                wn * BLK, wn * BLK, BLK)
                    ld_t = spool.tile([BLK, HEADS], f32, tag="ld")
                    nc.gpsimd.indirect_dma_start(
                        out=ld_t[:], out_offset=None, in_=ld1tab[:],
                        in_offset=IndirectOffsetOnAxis(ap=ldix_sb[:, b:b + 1], axis=0))
                    ldc = spool.tile([BLK, HEADS], f32, tag="ldc")
                    nc.vector.tensor_tensor(out=ldc[:], in0=ld_t[:], in1=csdb[:],
                                            op=mybir.AluOpType.add)
                    # ls = sum_c G*a1s  (grouped)
                    M = mpool.tile([BLK, wt * BLK], f32, tag="M")
                    M4 = M[:].rearrange("p (w h c) -> p w h c", h=HEADS, c=HID)
                    G4 = G3.rearrange("p w (h c) -> p w h c", c=HID)
                    a1s4 = a1sb[:].rearrange("p (h c) -> p h c", c=HID).unsqueeze(1)
                    nc.vector.tensor_tensor(out=M4, in0=G4,
                                            in1=a1s4.to_broadcast([BLK, wt, HEADS, HID]),
                                            op=mybir.AluOpType.mult)
                    lst = spool.tile([BLK, wt * HEADS], f32, tag="lst")
                    lst3 = lst[:].rearrange("p (w h) -> p w h", h=HEADS)
                    nc.vector.tensor_reduce(out=lst3, in_=M4,
                                            axis=mybir.AxisListType.X,
                                            op=mybir.AluOpType.add)
                    nc.vector.tensor_tensor(
                        out=lst3, in0=lst3,
                        in1=ldc[:].unsqueeze(1).to_broadcast([BLK, wt, HEADS]),
                        op=mybir.AluOpType.add)
                    nc.vector.tensor_tensor(
                        out=lst3, in0=lst3,
                        in1=mneg_sb[:, o:o + wt].unsqueeze(2).to_broadcast([BLK, wt, HEADS]),
                        op=mybir.AluOpType.add)
                    p_t = spool.tile([BLK, wt * HEADS], f32, tag="p")
                    p3 = p_t[:].rearrange("p (w h) -> p w h", h=HEADS)
                    nc.vector.tensor_scalar_mul(out=p_t[:], in0=lst[:], scalar1=NEG_SLOPE)
                    nc.vector.tensor_tensor(out=lst[:], in0=lst[:], in1=p_t[:],
                                            op=mybir.AluOpType.max)
                    den = spool.tile([BLK, HEADS], f32, tag="den")
                    for h in range(HEADS):
                        nc.scalar.activation(out=p3[:, :, h], in_=lst3[:, :, h],
                                             func=mybir.ActivationFunctionType.Exp,
                                             bias=kb1[:, 0:1],
                                             accum_out=den[:, h:h + 1])
                    nc.vector.tensor_scalar_add(out=den[:], in0=den[:], scalar1=1e-30)
                    # M = G * p
                    nc.vector.tensor_tensor(
                        out=M4, in0=G4,
                        in1=p3.unsqueeze(3).to_broadcast([BLK, wt, HEADS, HID]),
                        op=mybir.AluOpType.mult)
                    # tree reduce over w
                    M3 = M[:].rearrange("p (w f) -> p w f", f=BLK)
                    w = wt
                    while w > 1:
                        hsz = w // 2
                        nc.vector.tensor_tensor(out=M3[:, 0:hsz, :], in0=M3[:, 0:hsz, :],
                                                in1=M3[:, w - hsz:w, :],
                                                op=mybir.AluOpType.add)
                        w -= hsz
                    rcp = spool.tile([BLK, HEADS], f32, tag="rcp")
                    nc.vector.reciprocal(out=rcp[:], in_=den[:])
                    x2 = spool.tile([BLK, BLK], f32, tag="x2")
                    x2hc = x2[:].rearrange("p (h c) -> p h c", c=HID)
                    nc.vector.tensor_tensor(
                        out=x2hc, in0=M3[:, 0, :].rearrange("p (h c) -> p h c", c=HID),
                        in1=rcp[:].unsqueeze(2).to_broadcast([BLK, HEADS, HID]),
                        op=mybir.AluOpType.mult)
                    nc.vector.tensor_tensor(out=x2[:], in0=x2[:], in1=b1pb[:],
                                            op=mybir.AluOpType.add)
                    # elu
                    ex = spool.tile([BLK, BLK], f32, tag="ex")
                    nc.scalar.activation(out=ex[:], in_=x2[:],
                                         func=mybir.ActivationFunctionType.Exp)
                    nc.vector.tensor_scalar_add(out=ex[:], in0=ex[:], scalar1=-1.0)
                    nc.vector.tensor_scalar_min(out=ex[:], in0=ex[:], scalar1=0.0)
                    nc.vector.tensor_scalar_max(out=x2[:], in0=x2[:], scalar1=0.0)
                    nc.vector.tensor_tensor(out=x2[:], in0=x2[:], in1=ex[:],
                                            op=mybir.AluOpType.add)
                    # transpose -> slab
                    tps = pspool.tile([BLK, BLK], f32, tag="tps")
                    nc.tensor.transpose(out=tps[:], in_=x2[:], identity=ident[:])
                    x2T = spool.tile([BLK, BLK], f32, tag="x2T")
                    nc.vector.tensor_copy(out=x2T[:], in_=tps[:])
                    nc.sync.dma_start(out=x2slabT[:, b * BLK:(b + 1) * BLK], in_=x2T[:])

            # ---------------- AllGather x2 slabs
            if "3" in PHASES:
              nc.gpsimd.collective_compute(
                "AllGather", mybir.AluOpType.bypass,
                replica_groups=[list(range(NCORES))],
                ins=[x2slabT.opt()], outs=[x2fullT.opt()])

            # ---------------- P3: h2 / ld2 tables
            with (
                tc.tile_pool(name="p3", bufs=3) as pool,
                tc.tile_pool(name="p3ps", bufs=2, space="PSUM") as pspool,
            ):
                for t in range(NT if "3" in PHASES else 0):
                    sl = slice(t * BLK, (t + 1) * BLK)
                    c = t // NBLK
                    i0 = (t % NBLK) * BLK
                    xt = pool.tile([BLK, BLK], f32, tag="x2t")
                    nc.sync.dma_start(out=xt[:],
                                      in_=x2fullT[c * BLK:(c + 1) * BLK, i0:i0 + BLK])
                    ps = pspool.tile([BLK, 168], f32)
                    nc.tensor.matmul(out=ps[:], lhsT=xt[:], rhs=W2sb[:],
                                     start=True, stop=True)
                    hsb = pool.tile([BLK, 168], f32, tag="h2sb")
                    nc.vector.tensor_copy(out=hsb[:], in_=ps[:])
                    nc.sync.dma_start(out=h2tab[sl, 0:164], in_=hsb[:, 0:164])
                    nc.sync.dma_start(out=ld2tab[sl, :], in_=hsb[:, 164:168])

            # ---------------- P4: layer-2 aggregation -> out2
            with (
                tc.tile_pool(name="p4g", bufs=2) as gpool,
                tc.tile_pool(name="p4m", bufs=1) as mpool,
                tc.tile_pool(name="p4s", bufs=3) as spool,
            ):
                for b in range(NB_RUN if "4" in PHASES else 0):
                    wl, wh = int(Wlo[b]), int(Whi[b])
                    wt = wl + wh
                    o = int(offs[b])
                    G = gpool.tile([BLK, wt * TAB2], f32, tag="G2")
                    G3 = G[:].rearrange("p (w f) -> p w f", f=TAB2)
                    for (wbase, wlen, tab) in [(0, wl, h2tab[0:HALF, :]),
                                               (wl, wh, h2tab[HALF:NID, :])]:
                        for w0 in range(0, wlen, GCHUNK):
                            wn = min(GCHUNK, wlen - w0)
                            nc.gpsimd.dma_gather(
                                G3[:, wbase + w0:wbase + w0 + wn, :], tab,
                                idxw_sb[:, (o + wbase + w0) * 8:(o + wbase + w0 + wn) * 8],
                                wn * BLK, wn * BLK, TAB2)
                    ld_t = spool.tile([BLK, HEADS], f32, tag="ld2")
                    nc.gpsimd.indirect_dma_start(
                        out=ld_t[:], out_offset=None, in_=ld2tab[:],
                        in_offset=IndirectOffsetOnAxis(ap=ldix_sb[:, b:b + 1], axis=0))
                    lst = spool.tile([BLK, wt * HEADS], f32, tag="lst2")
                    lst3 = lst[:].rearrange("p (w h) -> p w h", h=HEADS)
                    nc.vector.tensor_tensor(
                        out=lst3, in0=G3[:, :, F2:F2 + HEADS],
                        in1=ld_t[:].unsqueeze(1).to_broadcast([BLK, wt, HEADS]),
                        op=mybir.AluOpType.add)
                    nc.vector.tensor_tensor(
                        out=lst3, in0=lst3,
                        in1=mneg_sb[:, o:o + wt].unsqueeze(2).to_broadcast([BLK, wt, HEADS]),
                        op=mybir.AluOpType.add)
                    p_t = spool.tile([BLK, wt * HEADS], f32, tag="p2")
                    p3 = p_t[:].rearrange("p (w h) -> p w h", h=HEADS)
                    nc.vector.tensor_scalar_mul(out=p_t[:], in0=lst[:], scalar1=NEG_SLOPE)
                    nc.vector.tensor_tensor(out=lst[:], in0=lst[:], in1=p_t[:],
                                            op=mybir.AluOpType.max)
                    den = spool.tile([BLK, HEADS], f32, tag="den2")
                    for h in range(HEADS):
                        nc.scalar.activation(out=p3[:, :, h], in_=lst3[:, :, h],
                                             func=mybir.ActivationFunctionType.Exp,
                                             bias=kb2[:, 0:1],
                                             accum_out=den[:, h:h + 1])
                    nc.vector.tensor_scalar_add(out=den[:], in0=den[:], scalar1=1e-30)
                    M = mpool.tile([BLK, wt * F2], f32, tag="M2")
                    M4 = M[:].rearrange("p (w h c) -> p w h c", h=HEADS, c=N_CLS)
                    G4 = G3[:, :, 0:F2].rearrange("p w (h c) -> p w h c", c=N_CLS)
                    nc.vector.tensor_tensor(
                        out=M4, in0=G4,
                        in1=p3.unsqueeze(3).to_broadcast([BLK, wt, HEADS, N_CLS]),
                        op=mybir.AluOpType.mult)
                    M3 = M[:].rearrange("p (w f) -> p w f", f=F2)
                    w = wt
                    while w > 1:
                        hsz = w // 2
                        nc.vector.tensor_tensor(out=M3[:, 0:hsz, :], in0=M3[:, 0:hsz, :],
                                                in1=M3[:, w - hsz:w, :],
                                                op=mybir.AluOpType.add)
                        w -= hsz
                    rcp = spool.tile([BLK, HEADS], f32, tag="rcp2")
                    nc.vector.reciprocal(out=rcp[:], in_=den[:])
                    ot = spool.tile([BLK, F2], f32, tag="ot")
                    nc.vector.tensor_tensor(
                        out=ot[:].rearrange("p (h c) -> p h c", c=N_CLS),
                        in0=M3[:, 0, :].rearrange("p (h c) -> p h c", c=N_CLS),
                        in1=rcp[:].unsqueeze(2).to_broadcast([BLK, HEADS, N_CLS]),
                        op=mybir.AluOpType.mult)
                    nc.sync.dma_start(out=t_out[b * BLK:(b + 1) * BLK, :], in_=ot[:])

    nc.compile()
    return nc


_CACHE = {}


def kernel(**inputs) -> np.ndarray:
    x = np.asarray(inputs["x"], np.float32)
    P = _prep_indices(np.asarray(inputs["edge_index"]))
    node_cid = P["node_cid"]

    fw = {}
    g = np.asarray(inputs["bn_gamma"], np.float32)
    bta = np.asarray(inputs["bn_beta"], np.float32)
    mu = np.asarray(inputs["bn_mean"], np.float32)
    var = np.asarray(inputs["bn_var"], np.float32)
    W1 = np.asarray(inputs["W1"], np.float32)
    a1s = np.asarray(inputs["a1_src"], np.float32)
    a1d = np.asarray(inputs["a1_dst"], np.float32)
    W2 = np.asarray(inputs["W2"], np.float32)
    a2s = np.asarray(inputs["a2_src"], np.float32)
    a2d = np.asarray(inputs["a2_dst"], np.float32)

    s = g / np.sqrt(var + BN_EPS)
    W1p = (s[:, None] * W1).astype(np.float32)
    b1p = ((bta - mu * s) @ W1).astype(np.float32)
    A1s = np.zeros((HEADS * HID, HEADS), np.float32)
    A1d = np.zeros((HEADS * HID, HEADS), np.float32)
    A2s = np.zeros((HEADS * N_CLS, HEADS), np.float32)
    A2d = np.zeros((HEADS * N_CLS, HEADS), np.float32)
    for h in range(HEADS):
        A1s[h * HID:(h + 1) * HID, h] = a1s[h]
        A1d[h * HID:(h + 1) * HID, h] = a1d[h]
        A2s[h * N_CLS:(h + 1) * N_CLS, h] = a2s[h]
        A2d[h * N_CLS:(h + 1) * N_CLS, h] = a2d[h]
    W1f = np.concatenate([W1p, W1p @ A1d], axis=1)            # [129, 132]
    csd = (b1p @ A1s + b1p @ A1d).astype(np.float32)
    W2f = np.concatenate([W2, W2 @ A2s, W2 @ A2d], axis=1).astype(np.float32)

    # x in cid space, transposed
    xp = np.zeros((NID, IN_F), np.float32)
    xp[node_cid] = x
    xT = np.ascontiguousarray(xp[:, :BLK].T)                  # [128, NID]
    xlast = np.ascontiguousarray(xp[:, BLK:BLK + 1])          # [NID, 1]

    key = (tuple(P["Wlo"]), tuple(P["Whi"]), os.environ.get("GAT_PHASES", "1234"), os.environ.get("GAT_NBLK", ""))
    if key not in _CACHE:
        _CACHE[key] = _build_program(P["Wlo"], P["Whi"], P["offs"], P["S"])
    nc = _CACHE[key]

    common = {
        "xT": xT, "xlast": xlast,
        "W1f": np.ascontiguousarray(W1f[:BLK]),
        "W1row": np.tile(W1f[BLK:BLK + 1], (BLK, 1)),
        "W2f": W2f,
        "b1pb": np.tile(b1p[None, :], (BLK, 1)),
        "csdb": np.tile(csd[None, :], (BLK, 1)),
    }
    # a1s as a flat [128]-vector: A1s is block-diagonal; its rowwise nonzero is
    # a1_src[h, c] at column h*HID+c -> flatten:
    a1flat = np.zeros(BLK, np.float32)
    for h in range(HEADS):
        a1flat[h * HID:(h + 1) * HID] = a1s[h]
    common["a1sb"] = np.tile(a1flat[None, :], (BLK, 1))

    in_maps = []
    for q in range(NCORES):
        m = dict(common)
        m["idxw"] = np.ascontiguousarray(P["idxw"][q])
        m["mneg"] = np.ascontiguousarray(P["maskneg"][q])
        m["ldidx"] = np.ascontiguousarray(P["ldidx"][q])
        in_maps.append(m)

    t0 = time.time()
    res = run_bass_kernel_spmd(nc, in_maps, core_ids=list(range(NCORES)))
    global last_run_seconds
    last_run_seconds = time.time() - t0
    outfull = np.concatenate([r["out2"] for r in res.results], axis=0)  # [NID, 160]
    return outfull[node_cid].astype(np.float32)


last_run_seconds = None
